# revision 1
# baseline (speedup 1.0000x reference)
"""Viterbi CRF decode on 8 Trainium2 NeuronCores.

Strategy: data-parallel over batch (32 sequences/core). The device kernel runs
the forward max-plus DP (alpha recurrence, the dominant compute) and streams the
full alpha history back to HBM. The host then does the O(L*B*T) backtrack over
that history (0.03% of the FLOPs) plus the sequence-length freeze handling.

Exactness: the device computes alpha_t[j] = max_i(fp32(alpha_{t-1}[i] +
trans[i,j])) + pot_t[j] with the same fp32 rounding as the jax reference, so the
backtrack argmax decisions (first-index tie-break) match bitwise.

Device layout per step (128 partitions = 4 j-quadrants x 32 sequences):
  vt[(q,b), (jb,i)] = alpha[b,i] + trans[i, 16q+jb]   (broadcast add, 1024/partition)
  m4[(q,b), jb]     = max_i vt                        (free-dim reduce)
  alpha'[b, 16q+jb] = m4[(q,b), jb] + pot             (4 collapse copies + add)
"""

import numpy as np

B, L, T = 256, 1024, 64
NCORES = 8
BC = B // NCORES  # 32 sequences per core
CH = 128          # potentials chunk (steps per DMA)

_cache = {}


def _build_program():
    if "nc" in _cache:
        return _cache["nc"]
    import concourse.bacc as bacc
    import concourse.mybir as mybir
    from concourse.tile import TileContext

    f32 = mybir.dt.float32
    AX = mybir.AxisListType
    OP = mybir.AluOpType

    nc = bacc.Bacc("TRN2", target_bir_lowering=False, debug=False)
    pots_in = nc.dram_tensor("pots", [BC, L, T], f32, kind="ExternalInput").ap()
    tsp_in = nc.dram_tensor("tspread", [128, 16, T], f32, kind="ExternalInput").ap()
    hist_out = nc.dram_tensor("ahist", [BC, L, T], f32, kind="ExternalOutput").ap()

    JBD = 12  # jb 0:12 added on DVE, 12:16 on Pool (DVE ~1.07, Pool ~3.0 ns/elem)

    with TileContext(nc) as tc:
        with tc.tile_pool(name="const", bufs=1) as cpool, \
             tc.tile_pool(name="pstream", bufs=2) as ppool, \
             tc.tile_pool(name="work", bufs=3) as wpool, \
             tc.tile_pool(name="big", bufs=1) as bpool:
            tsp = cpool.tile([128, 16, T], f32)
            nc.gpsimd.dma_start(out=tsp[:], in_=tsp_in[:])
            hist = bpool.tile([128, 256, T], f32)   # alpha history, 64KB/partition
            arep = cpool.tile([128, T], f32)

            nchunks = L // CH
            for c in range(nchunks):
                pc = ppool.tile([BC, CH, T], f32, tag="pots")
                nc.gpsimd.dma_start(out=pc[:], in_=pots_in[:, c * CH:(c + 1) * CH, :])

                if c == 0:
                    nc.vector.tensor_copy(arep[0:BC, :], pc[:, 0, :])
                    nc.gpsimd.tensor_copy(hist[0:BC, 0, :], arep[0:BC, :])
                    nc.vector.tensor_copy(arep[BC:2 * BC, :], arep[0:BC, :])
                    nc.vector.tensor_copy(arep[2 * BC:4 * BC, :], arep[0:2 * BC, :])

                t0 = max(c * CH, 1)
                for t in range(t0, (c + 1) * CH):
                    tg, tl = t >> 8, t & 255
                    s = t - c * CH
                    # vt[p, jb, i] = alpha[p%32, i] + trans[i, 16*(p//32)+jb]
                    vt = wpool.tile([128, 16, T], f32, tag="vt")
                    nc.vector.tensor_add(
                        vt[:, 0:JBD, :],
                        arep[:].unsqueeze(1).broadcast_to([128, JBD, T]),
                        tsp[:, 0:JBD, :],
                    )
                    nc.gpsimd.tensor_add(
                        vt[:, JBD:16, :],
                        arep[:].unsqueeze(1).broadcast_to([128, 16 - JBD, T]),
                        tsp[:, JBD:16, :],
                    )
                    m4 = wpool.tile([128, 16], f32, tag="m4")
                    nc.vector.tensor_reduce(m4[:], vt[:], axis=AX.X, op=OP.max)
                    ab = wpool.tile([BC, T], f32, tag="ab")
                    nc.vector.tensor_copy(ab[:, 0:16], m4[0:BC, :])
                    nc.gpsimd.tensor_copy(ab[:, 16:32], m4[BC:2 * BC, :])
                    nc.vector.tensor_copy(ab[:, 32:48], m4[2 * BC:3 * BC, :])
                    nc.gpsimd.tensor_copy(ab[:, 48:64], m4[3 * BC:4 * BC, :])
                    nc.vector.tensor_add(arep[0:BC, :], ab[:], pc[:, s, :])
                    nc.scalar.copy(hist[BC * tg:BC * (tg + 1), tl, :], arep[0:BC, :])
                    nc.vector.tensor_copy(arep[BC:2 * BC, :], arep[0:BC, :])
                    nc.gpsimd.tensor_copy(arep[2 * BC:3 * BC, :], arep[0:BC, :])
                    nc.vector.tensor_copy(arep[3 * BC:4 * BC, :], arep[0:BC, :])

            for tg in range(4):
                nc.gpsimd.dma_start(
                    out=hist_out[:, 256 * tg:256 * (tg + 1), :],
                    in_=hist[BC * tg:BC * (tg + 1), :, :],
                )

    nc.compile()
    _cache["nc"] = nc
    return nc


def _make_tspread(trans):
    # tsp[32q + b, jb, i] = trans[i, 16q + jb]
    tt = np.ascontiguousarray(trans.T).reshape(4, 16, T)  # [q, jb, i]
    return np.repeat(tt[:, None, :, :], BC, axis=1).reshape(128, 16, T).astype(np.float32)


def kernel(potentials, lengths, transition_params):
    from concourse.bass_utils import run_bass_kernel_spmd

    potentials = np.ascontiguousarray(np.asarray(potentials, dtype=np.float32))
    lengths = np.asarray(lengths, dtype=np.int32)
    trans = np.ascontiguousarray(np.asarray(transition_params, dtype=np.float32))

    nc = _build_program()
    tsp = _make_tspread(trans)
    in_maps = [
        {"pots": potentials[c * BC:(c + 1) * BC], "tspread": tsp}
        for c in range(NCORES)
    ]
    res = run_bass_kernel_spmd(nc, in_maps, core_ids=list(range(NCORES)))
    ah = np.concatenate([res.results[c]["ahist"] for c in range(NCORES)], axis=0)

    # Host backtrack over the device-computed alpha history.
    tags = np.zeros((B, L), dtype=np.int64)
    last = ah[np.arange(B), lengths - 1, :].argmax(axis=1)
    tags[:, L - 1] = last
    lm1 = lengths - 1
    for t in range(L - 2, -1, -1):
        nxt = tags[:, t + 1]
        cand = ah[:, t, :] + trans[:, nxt].T
        tags[:, t] = np.where(t >= lm1, last, cand.argmax(axis=1))
    return tags.astype(np.int32)



# revision 2
# speedup vs baseline: 4.7435x; 4.7435x over previous
"""Viterbi CRF decode on 8 Trainium2 NeuronCores — exp-domain PE formulation.

Strategy: data-parallel over batch (32 sequences/core). The forward max-plus DP
is run in the exponential domain so the TensorEngine does the heavy lifting:

    EZ_t[j,b]   = exp(K*(alpha_t[b,j] - n_t[b]))       (n_t arbitrary per-(b,t))
    V[j,b]      = sum_i expW[i,j] * EZ_{t-1}[i,b]      (PE matmul, fp32)
    EZ_t        = V * EP_t                             (DVE elementwise)

where expW = exp(K*trans) and EP_t[j,b] = exp(K*(pot[b,t,j]-max_j pot[b,t,j]))
are host-precomputed from the inputs. logsumexp/K approximates max within
log(64)/K; with K=64 the smooth-max bias is ~1e-4 vs typical decision margins
~0.1 (measured: 106/262144 tag flips, rel err 2.8e-4, gate is 2e-2).

The backtrack only compares values within one (b,t) slice, so any per-(b,t)
rescale of EZ is harmless: argmax_i(alpha[i]+trans[i,j]) == argmax_i
EZ[i]*expW[i,j] (monotone). Every G steps the state fed to the next matmul is
rescaled by 1/sum_i EZ (the sum comes free as a 65th ones-column in the
stationary weights; the per-b reciprocal is broadcast across partitions with a
K=1 matmul). The stored history keeps the unnormalized values.

Device per-step critical path: one 64x65x32 fp32 matmul + one [64,32] DVE
multiply writing straight into the history buffer (which is also the next
matmul's rhs). History streams back to HBM per 128-step chunk; the host does
the O(L*B*T) backtrack in float64 exp-domain products.
"""

import numpy as np

B, L, T = 256, 1024, 64
NCORES = 8
BC = B // NCORES   # 32 sequences per core
CH = 128           # steps per DMA chunk
KSC = 64.0         # exp-domain scale
G = 8              # renormalize every G steps
M = T + 1          # matmul output rows: 64 states + sum row

_cache = {}


def _build_program():
    if "nc" in _cache:
        return _cache["nc"]
    import concourse.bacc as bacc
    import concourse.mybir as mybir
    from concourse.tile import TileContext

    f32 = mybir.dt.float32

    nc = bacc.Bacc("TRN2", target_bir_lowering=False, debug=False)
    ep_in = nc.dram_tensor("ep", [T, L, BC], f32, kind="ExternalInput").ap()
    wext_in = nc.dram_tensor("wext", [T, M], f32, kind="ExternalInput").ap()
    hist_out = nc.dram_tensor("ezhist", [T, L, BC], f32, kind="ExternalOutput").ap()

    with TileContext(nc) as tc:
        with tc.tile_pool(name="const", bufs=1) as cpool, \
             tc.tile_pool(name="ep", bufs=2) as eppool, \
             tc.tile_pool(name="hist", bufs=1) as hpool, \
             tc.tile_pool(name="psv", bufs=4, space="PSUM") as pvpool, \
             tc.tile_pool(name="psb", bufs=2, space="PSUM") as pbpool:
            wext = cpool.tile([T, M], f32)
            nc.gpsimd.dma_start(out=wext[:], in_=wext_in[:])
            ones1 = cpool.tile([1, T], f32)
            nc.any.memset(ones1[:], 1.0)
            znorm = cpool.tile([T, BC], f32)
            rs = cpool.tile([1, BC], f32)
            hist = hpool.tile([T, L, BC], f32)   # 128KB/partition

            nchunks = L // CH
            for c in range(nchunks):
                ep = eppool.tile([T, CH, BC], f32, tag="ep")
                nc.gpsimd.dma_start(out=ep[:], in_=ep_in[:, c * CH:(c + 1) * CH, :])

                if c == 0:
                    nc.scalar.copy(hist[:, 0, :], ep[:, 0, :])

                t0 = max(c * CH, 1)
                for t in range(t0, (c + 1) * CH):
                    s = t - c * CH
                    tp = t - 1
                    if tp > 0 and tp % G == 0:
                        rhs = znorm[:]
                    else:
                        rhs = hist[:, tp, :]
                    v = pvpool.tile([M, BC], f32, tag="v")
                    nc.tensor.matmul(v[:], wext[:], rhs, start=True, stop=True)
                    nc.vector.tensor_mul(hist[:, t, :], v[0:T, :], ep[:, s, :])
                    if t % G == 0:
                        nc.vector.reciprocal(rs[:], v[T:M, :])
                        bc = pbpool.tile([T, BC], f32, tag="bc")
                        nc.tensor.matmul(bc[:], ones1[:], rs[:], start=True, stop=True)
                        nc.vector.tensor_mul(znorm[:], hist[:, t, :], bc[:])

                nc.gpsimd.dma_start(
                    out=hist_out[:, c * CH:(c + 1) * CH, :],
                    in_=hist[:, c * CH:(c + 1) * CH, :],
                )

    nc.compile()
    _cache["nc"] = nc
    return nc


def _host_precompute(potentials, trans):
    """EP per core ([T, L, BC] layout) and the extended stationary weights."""
    pm = potentials.max(axis=2, keepdims=True)
    EP = np.exp(KSC * (potentials - pm).astype(np.float64)).astype(np.float32)
    # [B, L, T] -> per-core [T, L, BC]
    eps = [np.ascontiguousarray(EP[c * BC:(c + 1) * BC].transpose(2, 1, 0))
           for c in range(NCORES)]
    expW = np.exp(KSC * trans.astype(np.float64)).astype(np.float32)   # [i, j]
    wext = np.concatenate([expW, np.ones((T, 1), np.float32)], axis=1)  # [T, M]
    return eps, wext, expW


def kernel(potentials, lengths, transition_params):
    from concourse.bass_utils import run_bass_kernel_spmd

    potentials = np.ascontiguousarray(np.asarray(potentials, dtype=np.float32))
    lengths = np.asarray(lengths, dtype=np.int32)
    trans = np.ascontiguousarray(np.asarray(transition_params, dtype=np.float32))

    nc = _build_program()
    eps, wext, expW = _host_precompute(potentials, trans)
    in_maps = [{"ep": eps[c], "wext": wext} for c in range(NCORES)]
    res = run_bass_kernel_spmd(nc, in_maps, core_ids=list(range(NCORES)))
    # [T, L, BC] per core -> EZ [B, L, T]
    EZ = np.concatenate(
        [res.results[c]["ezhist"].transpose(2, 1, 0) for c in range(NCORES)], axis=0)

    # Host backtrack in exp domain (monotone-equivalent to max-plus argmax).
    tags = np.zeros((B, L), dtype=np.int64)
    last = EZ[np.arange(B), lengths - 1, :].argmax(axis=1)
    tags[:, L - 1] = last
    lm1 = lengths - 1
    EW = expW.astype(np.float64)
    for t in range(L - 2, -1, -1):
        nxt = tags[:, t + 1]
        cand = EZ[:, t, :].astype(np.float64) * EW[:, nxt].T
        tags[:, t] = np.where(t >= lm1, last, cand.argmax(axis=1))
    return tags.astype(np.int32)


# revision 3
# speedup vs baseline: 6.7502x; 1.4230x over previous
"""Viterbi CRF decode on 8 Trainium2 NeuronCores — exp-domain PE formulation.

Strategy: data-parallel over batch (32 sequences/core). The forward max-plus DP
runs in the exponential domain so the TensorEngine does the heavy lifting:

    EZ_t[j,b]   = exp(K*(alpha_t[b,j] - n_t[b]))       (n_t arbitrary per-(b,t))
    V[j,b]      = sum_i expW[i,j] * EZ_{t-1}[i,b]      (PE matmul, bf16 in/fp32 acc)
    EZ_t        = V * EP_t                             (DVE elementwise, one op)

where expW = exp(K*trans) and EP_t[j,b] = exp(K*(pot[b,t,j]-max_j pot[b,t,j]))
are host-precomputed input transforms. logsumexp/K approximates max within
log(64)/K; with K=128 the measured error is ~55/262144 tag flips (rel 1.4e-4
vs the 2e-2 gate). bf16 quantization of EZ/W/EP adds ~6e-5 alpha noise per
step (products of bf16 are exact in the fp32 PSUM accumulate).

The backtrack only compares values within one (b,t) slice, so any per-(b,t)
rescale of EZ is harmless: argmax_i(alpha[i]+trans[i,j]) == argmax_i
EZ[i]*expW[i,j] (monotone). Every G steps the state fed into the next matmul
is rescaled by 1/sum_i EZ (sum via a tiny ones-column matmul, reciprocal on
DVE, partition-broadcast via a K=1 matmul); stored history stays unnormalized.

Device layout: the 32 sequences split into two 16-seq streams on partitions
0-63 / 64-127. The two per-step matmuls run CONCURRENTLY in disjoint PE
quadrants (tile_position (0,0) / (64,64)) writing disjoint halves of one PSUM
bank, so a single [128,16] DVE multiply serves both streams and writes
straight into the bf16 history buffer (which is also the next matmul's rhs).
History streams back to HBM per 128-step chunk; the host backtracks in f64.
"""

import numpy as np

B, L, T = 256, 1024, 64
NCORES = 8
BC = B // NCORES   # 32 sequences per core
HB = BC // 2       # 16 sequences per stream
CH = 128           # steps per DMA chunk
KSC = 128.0        # exp-domain scale
G = 16             # renormalize every G steps

_cache = {}


def _build_program():
    if "nc" in _cache:
        return _cache["nc"]
    import concourse.bacc as bacc
    import concourse.mybir as mybir
    from concourse.tile import TileContext

    f32 = mybir.dt.float32
    bf16 = mybir.dt.bfloat16

    nc = bacc.Bacc("TRN2", target_bir_lowering=False, debug=False)
    ep_in = nc.dram_tensor("ep", [128, L, HB], bf16, kind="ExternalInput").ap()
    wext_in = nc.dram_tensor("wext", [128, T], bf16, kind="ExternalInput").ap()
    hist_out = nc.dram_tensor("ezhist", [128, L, HB], bf16, kind="ExternalOutput").ap()

    with TileContext(nc) as tc:
        with tc.tile_pool(name="const", bufs=1) as cpool, \
             tc.tile_pool(name="ep", bufs=2) as eppool, \
             tc.tile_pool(name="hist", bufs=1) as hpool, \
             tc.tile_pool(name="psv", bufs=4, space="PSUM") as vpool, \
             tc.tile_pool(name="pss", bufs=2, space="PSUM") as spool:
            wext = cpool.tile([128, T], bf16)
            nc.gpsimd.dma_start(out=wext[:], in_=wext_in[:])
            onescol = cpool.tile([128, 1], bf16)
            nc.any.memset(onescol[:], 1.0)
            ones1 = cpool.tile([128, T], bf16)
            nc.any.memset(ones1[:], 1.0)
            rs32 = cpool.tile([128, HB], f32)
            rsb = cpool.tile([128, HB], bf16)
            znorm = cpool.tile([128, HB], bf16)
            hist = hpool.tile([128, L, HB], bf16)   # 32KB/partition

            nchunks = L // CH
            for c in range(nchunks):
                ep = eppool.tile([128, CH, HB], bf16, tag="ep")
                nc.gpsimd.dma_start(out=ep[:], in_=ep_in[:, c * CH:(c + 1) * CH, :])

                if c == 0:
                    nc.scalar.copy(hist[:, 0, :], ep[:, 0, :])

                t0 = max(c * CH, 1)
                for t in range(t0, (c + 1) * CH):
                    s = t - c * CH
                    tp = t - 1
                    renormed = tp > 0 and tp % G == 0
                    rhs = znorm if renormed else hist[:, tp, :]
                    vp = vpool.tile([128, HB], f32, tag="v")
                    nc.tensor.matmul(vp[0:64, :], wext[0:64, :], rhs[0:64, :],
                                     start=True, stop=True)
                    nc.tensor.matmul(vp[64:128, :], wext[64:128, :], rhs[64:128, :],
                                     start=True, stop=True)
                    nc.vector.tensor_mul(hist[:, t, :], vp[:], ep[:, s, :])
                    if t % G == 0:
                        sp = spool.tile([128, HB], f32, tag="s")
                        nc.tensor.matmul(sp[0:1, :], onescol[0:64, :],
                                         hist[0:64, t, :], start=True, stop=True)
                        nc.tensor.matmul(sp[64:65, :], onescol[64:128, :],
                                         hist[64:128, t, :], start=True, stop=True)
                        nc.vector.reciprocal(rs32[0:1, :], sp[0:1, :])
                        nc.vector.reciprocal(rs32[64:65, :], sp[64:65, :])
                        nc.scalar.copy(rsb[0:1, :], rs32[0:1, :])
                        nc.scalar.copy(rsb[64:65, :], rs32[64:65, :])
                        bcp = spool.tile([128, HB], f32, tag="bc")
                        nc.tensor.matmul(bcp[0:64, :], ones1[0:1, :], rsb[0:1, :],
                                         start=True, stop=True)
                        nc.tensor.matmul(bcp[64:128, :], ones1[64:65, :],
                                         rsb[64:65, :], start=True, stop=True)
                        nc.vector.tensor_mul(znorm[:], hist[:, t, :], bcp[:])

                nc.gpsimd.dma_start(
                    out=hist_out[:, c * CH:(c + 1) * CH, :],
                    in_=hist[:, c * CH:(c + 1) * CH, :],
                )

    nc.compile()
    _cache["nc"] = nc
    return nc


def _host_precompute(potentials, trans):
    """Per-core EP in [128, L, 16] bf16 stream layout + duplicated bf16 weights."""
    import ml_dtypes
    bf = ml_dtypes.bfloat16
    pm = potentials.max(axis=2, keepdims=True)
    EP = np.exp(KSC * (potentials - pm).astype(np.float64)).astype(bf)  # [B,L,T]
    eps = []
    for c in range(NCORES):
        epc = EP[c * BC:(c + 1) * BC]                    # [32, L, T]
        h = epc.reshape(2, HB, L, T).transpose(0, 3, 2, 1)  # [2, T, L, HB]
        eps.append(np.ascontiguousarray(h.reshape(128, L, HB)))
    expW = np.exp(KSC * trans.astype(np.float64)).astype(np.float32)    # [i, j]
    wext = np.ascontiguousarray(
        np.concatenate([expW, expW], axis=0).astype(bf))                # [128, T]
    return eps, wext, expW


def kernel(potentials, lengths, transition_params):
    from concourse.bass_utils import run_bass_kernel_spmd

    potentials = np.ascontiguousarray(np.asarray(potentials, dtype=np.float32))
    lengths = np.asarray(lengths, dtype=np.int32)
    trans = np.ascontiguousarray(np.asarray(transition_params, dtype=np.float32))

    nc = _build_program()
    eps, wext, expW = _host_precompute(potentials, trans)
    in_maps = [{"ep": eps[c], "wext": wext} for c in range(NCORES)]
    res = run_bass_kernel_spmd(nc, in_maps, core_ids=list(range(NCORES)))
    # [128, L, HB] per core -> EZ [B, L, T]
    parts = []
    for c in range(NCORES):
        arr = res.results[c]["ezhist"]                       # [128, L, HB] bf16
        h = arr.reshape(2, T, L, HB).transpose(0, 3, 2, 1)   # [2, HB, L, T]
        parts.append(h.reshape(BC, L, T))
    EZ = np.concatenate(parts, axis=0).astype(np.float64)    # [B, L, T]

    # Host backtrack in exp domain (monotone-equivalent to max-plus argmax).
    tags = np.zeros((B, L), dtype=np.int64)
    last = EZ[np.arange(B), lengths - 1, :].argmax(axis=1)
    tags[:, L - 1] = last
    lm1 = lengths - 1
    EW = expW.astype(np.float64)
    for t in range(L - 2, -1, -1):
        nxt = tags[:, t + 1]
        cand = EZ[:, t, :] * EW[:, nxt].T
        tags[:, t] = np.where(t >= lm1, last, cand.argmax(axis=1))
    return tags.astype(np.int32)


# revision 6
# speedup vs baseline: 7.1344x; 1.0569x over previous
"""Viterbi CRF decode on 8 Trainium2 NeuronCores — exp-domain PE formulation.

Strategy: data-parallel over batch (32 sequences/core). The forward max-plus DP
runs in the exponential domain so the TensorEngine does the heavy lifting:

    EZ_t[j,b]   = exp(K*(alpha_t[b,j] - n_t[b]))       (n_t arbitrary per-(b,t))
    V[j,b]      = sum_i expW[i,j] * EZ_{t-1}[i,b]      (PE matmul, bf16 in/fp32 acc)
    EZ_t        = V * EP_t                             (DVE elementwise, one op)

where expW = exp(K*trans) and EP_t[j,b] = exp(K*(pot[b,t,j]-max_j pot[b,t,j]))
are host-precomputed input transforms. logsumexp/K approximates max within
log(64)/K; with K=128 the measured error is ~55/262144 tag flips (rel 1.4e-4
vs the 2e-2 gate). bf16 quantization of EZ/W/EP adds ~6e-5 alpha noise per
step (products of bf16 are exact in the fp32 PSUM accumulate).

The backtrack only compares values within one (b,t) slice, so any per-(b,t)
rescale of EZ is harmless: argmax_i(alpha[i]+trans[i,j]) == argmax_i
EZ[i]*expW[i,j] (monotone). Every G steps the state fed into the next matmul
is rescaled by 1/sum_i EZ (sum via a tiny ones-column matmul, reciprocal on
DVE, partition-broadcast via a K=1 matmul); stored history stays unnormalized.

Device layout: the 32 sequences split into two 16-seq streams on partitions
0-63 / 64-127. The two per-step matmuls run CONCURRENTLY in disjoint PE
quadrants (tile_position (0,0) / (64,64)) writing disjoint halves of one PSUM
bank, so a single [128,16] DVE multiply serves both streams and writes
straight into the bf16 history buffer (which is also the next matmul's rhs).
History streams back to HBM per 128-step chunk; the host backtracks in f64.
"""

import numpy as np

B, L, T = 256, 1024, 64
NCORES = 8
BC = B // NCORES   # 32 sequences per core
HB = BC // 2       # 16 sequences per stream
CH = 128           # steps per DMA chunk
KSC = 128.0        # exp-domain scale
G = 8              # renormalize every G steps
LAG = 4            # renorm scale applied LAG steps after it is measured

_cache = {}


def _build_program():
    if "nc" in _cache:
        return _cache["nc"]
    import concourse.bacc as bacc
    import concourse.mybir as mybir
    from concourse.tile import TileContext

    f32 = mybir.dt.float32
    bf16 = mybir.dt.bfloat16

    nc = bacc.Bacc("TRN2", target_bir_lowering=False, debug=False)
    ep_in = nc.dram_tensor("ep", [128, L, HB], bf16, kind="ExternalInput").ap()
    wext_in = nc.dram_tensor("wext", [128, T], bf16, kind="ExternalInput").ap()
    hist_out = nc.dram_tensor("ezhist", [128, L, HB], bf16, kind="ExternalOutput").ap()

    with TileContext(nc) as tc:
        with tc.tile_pool(name="const", bufs=1) as cpool, \
             tc.tile_pool(name="ep", bufs=2) as eppool, \
             tc.tile_pool(name="hist", bufs=1) as hpool, \
             tc.tile_pool(name="psv", bufs=4, space="PSUM") as vpool, \
             tc.tile_pool(name="pss", bufs=2, space="PSUM") as spool:
            wext = cpool.tile([128, T], bf16)
            nc.gpsimd.dma_start(out=wext[:], in_=wext_in[:])
            onescol = cpool.tile([128, 1], bf16)
            nc.any.memset(onescol[:], 1.0)
            ones1 = cpool.tile([128, T], bf16)
            nc.any.memset(ones1[:], 1.0)
            rs32 = cpool.tile([128, HB], f32)
            rsb = cpool.tile([128, HB], bf16)
            eps1 = cpool.tile([128, HB], bf16)
            hist = hpool.tile([128, L, HB], bf16)   # 32KB/partition

            nchunks = L // CH
            for c in range(nchunks):
                ep = eppool.tile([128, CH, HB], bf16, tag="ep")
                nc.gpsimd.dma_start(out=ep[:], in_=ep_in[:, c * CH:(c + 1) * CH, :])

                if c == 0:
                    nc.scalar.copy(hist[:, 0, :], ep[:, 0, :])

                t0 = max(c * CH, 1)
                for t in range(t0, (c + 1) * CH):
                    s = t - c * CH
                    vp = vpool.tile([128, HB], f32, tag="v")
                    nc.tensor.matmul(vp[0:64, :], wext[0:64, :],
                                     hist[0:64, t - 1, :], start=True, stop=True)
                    nc.tensor.matmul(vp[64:128, :], wext[64:128, :],
                                     hist[64:128, t - 1, :], start=True, stop=True)
                    # The renorm scale is pre-folded into eps1 for apply steps,
                    # so the serial chain is identical every step.
                    use_eps1 = t % G == LAG and t >= G + LAG
                    src1 = eps1[:] if use_eps1 else ep[:, s, :]
                    nc.vector.tensor_mul(hist[:, t, :], vp[:], src1)
                    if t % G == 0 and G <= t < L - LAG:
                        # Off-chain renorm: S = sum_i EZ_t, scale 1/S folded
                        # into the EP slice of step t+LAG (same DMA chunk).
                        sp = spool.tile([128, HB], f32, tag="s")
                        nc.tensor.matmul(sp[0:1, :], onescol[0:64, :],
                                         hist[0:64, t, :], start=True, stop=True)
                        nc.tensor.matmul(sp[64:65, :], onescol[64:128, :],
                                         hist[64:128, t, :], start=True, stop=True)
                        nc.vector.reciprocal(rs32[0:1, :], sp[0:1, :])
                        nc.vector.reciprocal(rs32[64:65, :], sp[64:65, :])
                        nc.scalar.copy(rsb[0:1, :], rs32[0:1, :])
                        nc.scalar.copy(rsb[64:65, :], rs32[64:65, :])
                        bcp = spool.tile([128, HB], f32, tag="bc")
                        nc.tensor.matmul(bcp[0:64, :], ones1[0:1, :], rsb[0:1, :],
                                         start=True, stop=True)
                        nc.tensor.matmul(bcp[64:128, :], ones1[64:65, :],
                                         rsb[64:65, :], start=True, stop=True)
                        nc.vector.tensor_mul(eps1[:], ep[:, s + LAG, :], bcp[:])

                nc.gpsimd.dma_start(
                    out=hist_out[:, c * CH:(c + 1) * CH, :],
                    in_=hist[:, c * CH:(c + 1) * CH, :],
                )

    nc.compile()
    _cache["nc"] = nc
    return nc


def _host_precompute(potentials, trans):
    """Per-core EP in [128, L, 16] bf16 stream layout + duplicated bf16 weights."""
    import ml_dtypes
    bf = ml_dtypes.bfloat16
    pm = potentials.max(axis=2, keepdims=True)
    EP = np.exp(KSC * (potentials - pm).astype(np.float64)).astype(bf)  # [B,L,T]
    eps = []
    for c in range(NCORES):
        epc = EP[c * BC:(c + 1) * BC]                    # [32, L, T]
        h = epc.reshape(2, HB, L, T).transpose(0, 3, 2, 1)  # [2, T, L, HB]
        eps.append(np.ascontiguousarray(h.reshape(128, L, HB)))
    expW = np.exp(KSC * trans.astype(np.float64)).astype(np.float32)    # [i, j]
    wext = np.ascontiguousarray(
        np.concatenate([expW, expW], axis=0).astype(bf))                # [128, T]
    return eps, wext, expW


def kernel(potentials, lengths, transition_params):
    from concourse.bass_utils import run_bass_kernel_spmd

    potentials = np.ascontiguousarray(np.asarray(potentials, dtype=np.float32))
    lengths = np.asarray(lengths, dtype=np.int32)
    trans = np.ascontiguousarray(np.asarray(transition_params, dtype=np.float32))

    nc = _build_program()
    eps, wext, expW = _host_precompute(potentials, trans)
    in_maps = [{"ep": eps[c], "wext": wext} for c in range(NCORES)]
    res = run_bass_kernel_spmd(nc, in_maps, core_ids=list(range(NCORES)))
    # [128, L, HB] per core -> EZ [B, L, T]
    parts = []
    for c in range(NCORES):
        arr = res.results[c]["ezhist"]                       # [128, L, HB] bf16
        h = arr.reshape(2, T, L, HB).transpose(0, 3, 2, 1)   # [2, HB, L, T]
        parts.append(h.reshape(BC, L, T))
    EZ = np.concatenate(parts, axis=0).astype(np.float64)    # [B, L, T]

    # Host backtrack in exp domain (monotone-equivalent to max-plus argmax).
    tags = np.zeros((B, L), dtype=np.int64)
    last = EZ[np.arange(B), lengths - 1, :].argmax(axis=1)
    tags[:, L - 1] = last
    lm1 = lengths - 1
    EW = expW.astype(np.float64)
    for t in range(L - 2, -1, -1):
        nxt = tags[:, t + 1]
        cand = EZ[:, t, :] * EW[:, nxt].T
        tags[:, t] = np.where(t >= lm1, last, cand.argmax(axis=1))
    return tags.astype(np.int32)


# revision 11
# speedup vs baseline: 9.2496x; 1.2965x over previous
"""Viterbi CRF decode on 8 Trainium2 NeuronCores — exp-domain PE formulation.

Strategy: data-parallel over batch (32 sequences/core). The forward max-plus DP
runs in the exponential domain so the TensorEngine does the heavy lifting:

    EZ_t[j,b]   = exp(K*(alpha_t[b,j] - n_t[b]))       (n_t arbitrary per-(b,t))
    V[j,b]      = sum_i expW[i,j] * EZ_{t-1}[i,b]      (PE matmul, bf16 in/fp32 acc)
    EZ_t        = V * EP_t                             (DVE elementwise, one op)

where expW = exp(K*trans) and EP_t[j,b] = exp(K*(pot[b,t,j]-max_j pot[b,t,j]))
are host-precomputed input transforms. logsumexp/K approximates max within
log(64)/K; with K=128 the measured error is ~34/262144 tag flips (rel ~1e-4
vs the 2e-2 gate). bf16 quantization of EZ/W/EP adds ~6e-5 alpha noise per
step (products of bf16 are exact in the fp32 PSUM accumulate).

The backtrack only compares values within one (b,t) slice, so any per-(b,t)
rescale of EZ is harmless: argmax_i(alpha[i]+trans[i,j]) == argmax_i
EZ[i]*expW[i,j] (monotone). Every G steps a 1/sum_i EZ rescale (per stream) is
folded off-critical-path into the EP slice of step t+LAG, so the serial chain
is structurally identical every step: one matmul + one DVE multiply.

Device layout: 32 sequences split into two 16-seq streams on partitions 0-63 /
64-127 with BLOCK-DIAGONAL weights [128,128], so ONE matmul per step serves
both streams (K=M=128, N=16), writing one PSUM bank that a single [128,16]
DVE multiply turns into the next bf16 state, written straight into the
history buffer (also the next matmul's rhs). The S-row sums use a [128,2]
selector matmul; the per-b broadcast of 1/S uses a K=2 mask matmul. A one-time
burst of dummy matmuls at startup locks the PE HAM clock-gate at 2.4 GHz
(steady-state gaps are too short to ever re-throttle it).

History streams back to HBM per 128-step chunk; the host backtracks in f64.
"""

import numpy as np

B, L, T = 256, 1024, 64
NCORES = 8
BC = B // NCORES   # 32 sequences per core
HB = BC // 2       # 16 sequences per stream
CH = 128           # steps per DMA chunk
KSC = 128.0        # exp-domain scale
G = 16             # renormalize every G steps
LAG = 4            # renorm scale applied LAG steps after it is measured
NWARM = 100        # startup dummy matmuls to warm the PE HAM clock gate

_cache = {}


def _build_program():
    if "nc" in _cache:
        return _cache["nc"]
    import concourse.bacc as bacc
    import concourse.mybir as mybir
    from concourse.tile import TileContext

    f32 = mybir.dt.float32
    bf16 = mybir.dt.bfloat16

    nc = bacc.Bacc("TRN2", target_bir_lowering=False, debug=False)
    ep_in = nc.dram_tensor("ep", [128, L, HB], bf16, kind="ExternalInput").ap()
    w_in = nc.dram_tensor("wbig", [128, 128], bf16, kind="ExternalInput").ap()
    scol_in = nc.dram_tensor("scol", [128, 2], bf16, kind="ExternalInput").ap()
    bmask_in = nc.dram_tensor("bmask", [2, 128], bf16, kind="ExternalInput").ap()
    hist_out = nc.dram_tensor("ezhist", [128, L, HB], bf16, kind="ExternalOutput").ap()

    with TileContext(nc) as tc:
        with tc.tile_pool(name="const", bufs=1) as cpool, \
             tc.tile_pool(name="ep", bufs=2) as eppool, \
             tc.tile_pool(name="hist", bufs=1) as hpool, \
             tc.tile_pool(name="psv", bufs=4, space="PSUM") as vpool, \
             tc.tile_pool(name="pss", bufs=1, space="PSUM") as spool, \
             tc.tile_pool(name="warm", bufs=1, space="PSUM") as wpool:
            wbig = cpool.tile([128, 128], bf16)
            nc.gpsimd.dma_start(out=wbig[:], in_=w_in[:])
            scol = cpool.tile([128, 2], bf16)
            nc.gpsimd.dma_start(out=scol[:], in_=scol_in[:])
            bmask = cpool.tile([2, 128], bf16)
            nc.gpsimd.dma_start(out=bmask[:], in_=bmask_in[:])
            rs32 = cpool.tile([2, HB], f32)
            rsb = cpool.tile([2, HB], bf16)
            eps1 = cpool.tile([128, HB], bf16)
            hist = hpool.tile([128, L, HB], bf16)   # 32KB/partition

            # One-time PE warmup: ~NWARM back-to-back dummy matmuls (~4us busy)
            # flip the HAM clock gate to K=8/8; the per-step gaps afterwards
            # are far below the ~3.4us idle window, so it never re-throttles.
            warm = wpool.tile([128, 64], f32)
            for _ in range(NWARM):
                nc.tensor.matmul(warm[:], wbig[:], wbig[:, 0:64],
                                 start=True, stop=True)

            nchunks = L // CH
            for c in range(nchunks):
                ep = eppool.tile([128, CH, HB], bf16, tag="ep")
                nc.gpsimd.dma_start(out=ep[:], in_=ep_in[:, c * CH:(c + 1) * CH, :])

                if c == 0:
                    nc.scalar.copy(hist[:, 0, :], ep[:, 0, :])

                t0 = max(c * CH, 1)
                for t in range(t0, (c + 1) * CH):
                    s = t - c * CH
                    vp = vpool.tile([128, HB], f32, tag="v")
                    nc.tensor.matmul(vp[:], wbig[:], hist[:, t - 1, :],
                                     start=True, stop=True)
                    # The renorm scale is pre-folded into eps1 for apply steps,
                    # so the serial chain is identical every step.
                    use_eps1 = t % G == LAG and t >= G + LAG
                    src1 = eps1[:] if use_eps1 else ep[:, s, :]
                    nc.vector.tensor_mul(hist[:, t, :], vp[:], src1)
                    if t % G == 0 and G <= t < L - LAG:
                        # Off-chain renorm: S = per-stream sum_i EZ_t (rows 0/1
                        # via the selector matmul), scale 1/S broadcast with a
                        # K=2 mask matmul, folded into EP of step t+LAG.
                        sp = spool.tile([2, HB], f32, tag="s")
                        nc.tensor.matmul(sp[:], scol[:], hist[:, t, :],
                                         start=True, stop=True)
                        nc.vector.reciprocal(rs32[:], sp[:])
                        nc.scalar.copy(rsb[:], rs32[:])
                        bcp = spool.tile([128, HB], f32, tag="bc")
                        nc.tensor.matmul(bcp[:], bmask[:], rsb[:],
                                         start=True, stop=True)
                        nc.vector.tensor_mul(eps1[:], ep[:, s + LAG, :], bcp[:])

                nc.gpsimd.dma_start(
                    out=hist_out[:, c * CH:(c + 1) * CH, :],
                    in_=hist[:, c * CH:(c + 1) * CH, :],
                )

    nc.compile()
    _cache["nc"] = nc
    return nc


def _host_precompute(potentials, trans):
    """Per-core EP in [128, L, 16] bf16 stream layout + block-diag weights."""
    import ml_dtypes
    bf = ml_dtypes.bfloat16
    pm = potentials.max(axis=2, keepdims=True)
    EP = np.exp(KSC * (potentials - pm).astype(np.float64)).astype(bf)  # [B,L,T]
    eps = []
    for c in range(NCORES):
        epc = EP[c * BC:(c + 1) * BC]                    # [32, L, T]
        h = epc.reshape(2, HB, L, T).transpose(0, 3, 2, 1)  # [2, T, L, HB]
        eps.append(np.ascontiguousarray(h.reshape(128, L, HB)))
    expW = np.exp(KSC * trans.astype(np.float64)).astype(np.float32)    # [i, j]
    wbig = np.zeros((128, 128), dtype=bf)
    wbig[0:T, 0:T] = expW.astype(bf)
    wbig[T:128, T:128] = expW.astype(bf)
    scol = np.zeros((128, 2), dtype=bf)
    scol[0:T, 0] = 1
    scol[T:128, 1] = 1
    bmask = np.zeros((2, 128), dtype=bf)
    bmask[0, 0:T] = 1
    bmask[1, T:128] = 1
    return eps, wbig, scol, bmask, expW


def kernel(potentials, lengths, transition_params):
    from concourse.bass_utils import run_bass_kernel_spmd

    potentials = np.ascontiguousarray(np.asarray(potentials, dtype=np.float32))
    lengths = np.asarray(lengths, dtype=np.int32)
    trans = np.ascontiguousarray(np.asarray(transition_params, dtype=np.float32))

    nc = _build_program()
    eps, wbig, scol, bmask, expW = _host_precompute(potentials, trans)
    in_maps = [{"ep": eps[c], "wbig": wbig, "scol": scol, "bmask": bmask}
               for c in range(NCORES)]
    res = run_bass_kernel_spmd(nc, in_maps, core_ids=list(range(NCORES)))
    # [128, L, HB] per core -> EZ [B, L, T]
    parts = []
    for c in range(NCORES):
        arr = res.results[c]["ezhist"]                       # [128, L, HB] bf16
        h = arr.reshape(2, T, L, HB).transpose(0, 3, 2, 1)   # [2, HB, L, T]
        parts.append(h.reshape(BC, L, T))
    EZ = np.concatenate(parts, axis=0).astype(np.float64)    # [B, L, T]

    # Host backtrack in exp domain (monotone-equivalent to max-plus argmax).
    tags = np.zeros((B, L), dtype=np.int64)
    last = EZ[np.arange(B), lengths - 1, :].argmax(axis=1)
    tags[:, L - 1] = last
    lm1 = lengths - 1
    EW = expW.astype(np.float64)
    for t in range(L - 2, -1, -1):
        nxt = tags[:, t + 1]
        cand = EZ[:, t, :] * EW[:, nxt].T
        tags[:, t] = np.where(t >= lm1, last, cand.argmax(axis=1))
    return tags.astype(np.int32)


# revision 14
# speedup vs baseline: 9.2543x; 1.0005x over previous
"""Viterbi CRF decode on 8 Trainium2 NeuronCores — exp-domain PE formulation.

Strategy: data-parallel over batch (32 sequences/core). The forward max-plus DP
runs in the exponential domain so the TensorEngine does the heavy lifting:

    EZ_t[j,b]   = exp(K*(alpha_t[b,j] - n_t[b]))       (n_t arbitrary per-(b,t))
    V[j,b]      = sum_i expW[i,j] * EZ_{t-1}[i,b]      (PE matmul, bf16 in/fp32 acc)
    EZ_t        = V * EP_t                             (DVE elementwise, one op)

where expW = exp(K*trans) and EP_t[j,b] = exp(K*(pot[b,t,j]-max_j pot[b,t,j]))
are host-precomputed input transforms. logsumexp/K approximates max within
log(64)/K; with K=128 the measured error is ~34/262144 tag flips (rel ~1e-4
vs the 2e-2 gate). bf16 quantization of EZ/W/EP adds ~6e-5 alpha noise per
step (products of bf16 are exact in the fp32 PSUM accumulate).

The backtrack only compares values within one (b,t) slice, so any per-(b,t)
rescale of EZ is harmless: argmax_i(alpha[i]+trans[i,j]) == argmax_i
EZ[i]*expW[i,j] (monotone). Every G steps a 1/sum_i EZ rescale (per stream) is
folded off-critical-path into the EP slice of step t+LAG, so the serial chain
is structurally identical every step: one matmul + one DVE multiply.

Device layout: 32 sequences split into two 16-seq streams on partitions 0-63 /
64-127 with BLOCK-DIAGONAL weights [128,128], so ONE matmul per step serves
both streams (K=M=128, N=16), writing one PSUM bank that a single [128,16]
DVE multiply turns into the next bf16 state, written straight into the
history buffer (also the next matmul's rhs). The S-row sums use a [128,2]
selector matmul; the per-b broadcast of 1/S uses a K=2 mask matmul. A one-time
burst of dummy matmuls at startup locks the PE HAM clock-gate at 2.4 GHz
(steady-state gaps are too short to ever re-throttle it).

History streams back to HBM per 128-step chunk; the host backtracks in f64.
"""

import numpy as np

B, L, T = 256, 1024, 64
NCORES = 8
BC = B // NCORES   # 32 sequences per core
HB = BC // 2       # 16 sequences per stream
CH = 128           # steps per DMA chunk
KSC = 128.0        # exp-domain scale
G = 16             # renormalize every G steps
LAG = 4            # renorm scale applied LAG steps after it is measured
NWARM = 60         # startup dummy matmuls to warm the PE HAM clock gate

_cache = {}


def _build_program():
    if "nc" in _cache:
        return _cache["nc"]
    import concourse.bacc as bacc
    import concourse.mybir as mybir
    from concourse.tile import TileContext

    f32 = mybir.dt.float32
    bf16 = mybir.dt.bfloat16

    nc = bacc.Bacc("TRN2", target_bir_lowering=False, debug=False)
    ep_in = nc.dram_tensor("ep", [128, L, HB], bf16, kind="ExternalInput").ap()
    w_in = nc.dram_tensor("wbig", [128, 128], bf16, kind="ExternalInput").ap()
    scol_in = nc.dram_tensor("scol", [128, 2], bf16, kind="ExternalInput").ap()
    bmask_in = nc.dram_tensor("bmask", [2, 128], bf16, kind="ExternalInput").ap()
    hist_out = nc.dram_tensor("ezhist", [128, L, HB], bf16, kind="ExternalOutput").ap()

    with TileContext(nc) as tc:
        with tc.tile_pool(name="const", bufs=1) as cpool, \
             tc.tile_pool(name="ep", bufs=2) as eppool, \
             tc.tile_pool(name="hist", bufs=1) as hpool, \
             tc.tile_pool(name="psv", bufs=4, space="PSUM") as vpool, \
             tc.tile_pool(name="pss", bufs=1, space="PSUM") as spool, \
             tc.tile_pool(name="warm", bufs=1, space="PSUM") as wpool:
            wbig = cpool.tile([128, 128], bf16)
            nc.gpsimd.dma_start(out=wbig[:], in_=w_in[:])
            scol = cpool.tile([128, 2], bf16)
            nc.gpsimd.dma_start(out=scol[:], in_=scol_in[:])
            bmask = cpool.tile([2, 128], bf16)
            nc.gpsimd.dma_start(out=bmask[:], in_=bmask_in[:])
            rs32 = cpool.tile([2, HB], f32)
            rsb = cpool.tile([2, HB], bf16)
            eps1 = cpool.tile([128, HB], bf16)
            hist = hpool.tile([128, L, HB], bf16)   # 32KB/partition

            # One-time PE warmup: ~NWARM back-to-back dummy matmuls (~4-5us of
            # continuous PE activity) flip the HAM clock gate to K=8/8; the
            # per-step gaps afterwards are far below the ~3.4us idle window,
            # so it never re-throttles. A memset source avoids any DMA
            # dependency, so warmup overlaps the input DMAs.
            wsrc = cpool.tile([128, 64], bf16)
            nc.any.memset(wsrc[:], 1.0)
            warm = wpool.tile([64, 64], f32)
            for _ in range(NWARM):
                nc.tensor.matmul(warm[:], wsrc[:], wsrc[:], start=True, stop=True)

            nchunks = L // CH
            for c in range(nchunks):
                ep = eppool.tile([128, CH, HB], bf16, tag="ep")
                nc.gpsimd.dma_start(out=ep[:], in_=ep_in[:, c * CH:(c + 1) * CH, :])

                if c == 0:
                    nc.scalar.copy(hist[:, 0, :], ep[:, 0, :])

                t0 = max(c * CH, 1)
                for t in range(t0, (c + 1) * CH):
                    s = t - c * CH
                    vp = vpool.tile([128, HB], f32, tag="v")
                    nc.tensor.matmul(vp[:], wbig[:], hist[:, t - 1, :],
                                     start=True, stop=True)
                    # The renorm scale is pre-folded into eps1 for apply steps,
                    # so the serial chain is identical every step.
                    use_eps1 = t % G == LAG and t >= G + LAG
                    src1 = eps1[:] if use_eps1 else ep[:, s, :]
                    nc.vector.tensor_mul(hist[:, t, :], vp[:], src1)
                    if t % G == 0 and G <= t < L - LAG:
                        # Off-chain renorm: S = per-stream sum_i EZ_t (rows 0/1
                        # via the selector matmul), scale 1/S broadcast with a
                        # K=2 mask matmul, folded into EP of step t+LAG.
                        sp = spool.tile([2, HB], f32, tag="s")
                        nc.tensor.matmul(sp[:], scol[:], hist[:, t, :],
                                         start=True, stop=True)
                        nc.vector.reciprocal(rs32[:], sp[:])
                        nc.scalar.copy(rsb[:], rs32[:])
                        bcp = spool.tile([128, HB], f32, tag="bc")
                        nc.tensor.matmul(bcp[:], bmask[:], rsb[:],
                                         start=True, stop=True)
                        nc.vector.tensor_mul(eps1[:], ep[:, s + LAG, :], bcp[:])

                nc.gpsimd.dma_start(
                    out=hist_out[:, c * CH:(c + 1) * CH, :],
                    in_=hist[:, c * CH:(c + 1) * CH, :],
                )

    nc.compile()
    _cache["nc"] = nc
    return nc


def _host_precompute(potentials, trans):
    """Per-core EP in [128, L, 16] bf16 stream layout + block-diag weights."""
    import ml_dtypes
    bf = ml_dtypes.bfloat16
    pm = potentials.max(axis=2, keepdims=True)
    EP = np.exp(KSC * (potentials - pm).astype(np.float64)).astype(bf)  # [B,L,T]
    eps = []
    for c in range(NCORES):
        epc = EP[c * BC:(c + 1) * BC]                    # [32, L, T]
        h = epc.reshape(2, HB, L, T).transpose(0, 3, 2, 1)  # [2, T, L, HB]
        eps.append(np.ascontiguousarray(h.reshape(128, L, HB)))
    expW = np.exp(KSC * trans.astype(np.float64)).astype(np.float32)    # [i, j]
    wbig = np.zeros((128, 128), dtype=bf)
    wbig[0:T, 0:T] = expW.astype(bf)
    wbig[T:128, T:128] = expW.astype(bf)
    scol = np.zeros((128, 2), dtype=bf)
    scol[0:T, 0] = 1
    scol[T:128, 1] = 1
    bmask = np.zeros((2, 128), dtype=bf)
    bmask[0, 0:T] = 1
    bmask[1, T:128] = 1
    return eps, wbig, scol, bmask, expW


def kernel(potentials, lengths, transition_params):
    from concourse.bass_utils import run_bass_kernel_spmd

    potentials = np.ascontiguousarray(np.asarray(potentials, dtype=np.float32))
    lengths = np.asarray(lengths, dtype=np.int32)
    trans = np.ascontiguousarray(np.asarray(transition_params, dtype=np.float32))

    nc = _build_program()
    eps, wbig, scol, bmask, expW = _host_precompute(potentials, trans)
    in_maps = [{"ep": eps[c], "wbig": wbig, "scol": scol, "bmask": bmask}
               for c in range(NCORES)]
    res = run_bass_kernel_spmd(nc, in_maps, core_ids=list(range(NCORES)))
    # [128, L, HB] per core -> EZ [B, L, T]
    parts = []
    for c in range(NCORES):
        arr = res.results[c]["ezhist"]                       # [128, L, HB] bf16
        h = arr.reshape(2, T, L, HB).transpose(0, 3, 2, 1)   # [2, HB, L, T]
        parts.append(h.reshape(BC, L, T))
    EZ = np.concatenate(parts, axis=0).astype(np.float64)    # [B, L, T]

    # Host backtrack in exp domain (monotone-equivalent to max-plus argmax).
    tags = np.zeros((B, L), dtype=np.int64)
    last = EZ[np.arange(B), lengths - 1, :].argmax(axis=1)
    tags[:, L - 1] = last
    lm1 = lengths - 1
    EW = expW.astype(np.float64)
    for t in range(L - 2, -1, -1):
        nxt = tags[:, t + 1]
        cand = EZ[:, t, :] * EW[:, nxt].T
        tags[:, t] = np.where(t >= lm1, last, cand.argmax(axis=1))
    return tags.astype(np.int32)


# revision 17
# speedup vs baseline: 9.2552x; 1.0001x over previous
"""Viterbi CRF decode on 8 Trainium2 NeuronCores — exp-domain PE formulation.

Strategy: data-parallel over batch (32 sequences/core). The forward max-plus DP
runs in the exponential domain so the TensorEngine does the heavy lifting:

    EZ_t[j,b]   = exp(K*(alpha_t[b,j] - n_t[b]))       (n_t arbitrary per-(b,t))
    V[j,b]      = sum_i expW[i,j] * EZ_{t-1}[i,b]      (PE matmul, bf16 in/fp32 acc)
    EZ_t        = V * EP_t                             (DVE elementwise, one op)

where expW = exp(K*trans) and EP_t[j,b] = exp(K*(pot[b,t,j]-max_j pot[b,t,j]))
are host-precomputed input transforms. logsumexp/K approximates max within
log(64)/K; with K=128 the measured error is ~34/262144 tag flips (rel ~1e-4
vs the 2e-2 gate). bf16 quantization of EZ/W/EP adds ~6e-5 alpha noise per
step (products of bf16 are exact in the fp32 PSUM accumulate).

The backtrack only compares values within one (b,t) slice, so any per-(b,t)
rescale of EZ is harmless: argmax_i(alpha[i]+trans[i,j]) == argmax_i
EZ[i]*expW[i,j] (monotone). Every G steps a 1/sum_i EZ rescale (per stream) is
folded off-critical-path into the EP slice of step t+LAG, so the serial chain
is structurally identical every step: one matmul + one DVE multiply.

Device layout: 32 sequences split into two 16-seq streams on partitions 0-63 /
64-127 with BLOCK-DIAGONAL weights [128,128], so ONE matmul per step serves
both streams (K=M=128, N=16), writing one PSUM bank that a single [128,16]
DVE multiply turns into the next bf16 state, written straight into the
history buffer (also the next matmul's rhs). The S-row sums use a [128,2]
selector matmul; the per-b broadcast of 1/S uses a K=2 mask matmul. A one-time
burst of dummy matmuls at startup locks the PE HAM clock-gate at 2.4 GHz
(steady-state gaps are too short to ever re-throttle it).

History streams back to HBM per 128-step chunk; the host backtracks in f64.
"""

import numpy as np

B, L, T = 256, 1024, 64
NCORES = 8
BC = B // NCORES   # 32 sequences per core
HB = BC // 2       # 16 sequences per stream
CH = 128           # steps per DMA chunk
KSC = 128.0        # exp-domain scale
G = 16             # renormalize every G steps
LAG = 4            # renorm scale applied LAG steps after it is measured
NWARM = 40         # startup dummy matmuls to warm the PE HAM clock gate

_cache = {}


def _build_program():
    if "nc" in _cache:
        return _cache["nc"]
    import concourse.bacc as bacc
    import concourse.mybir as mybir
    from concourse.tile import TileContext

    f32 = mybir.dt.float32
    bf16 = mybir.dt.bfloat16

    nc = bacc.Bacc("TRN2", target_bir_lowering=False, debug=False)
    ep_in = nc.dram_tensor("ep", [128, L, HB], bf16, kind="ExternalInput").ap()
    w_in = nc.dram_tensor("wbig", [128, 128], bf16, kind="ExternalInput").ap()
    scol_in = nc.dram_tensor("scol", [128, 2], bf16, kind="ExternalInput").ap()
    bmask_in = nc.dram_tensor("bmask", [2, 128], bf16, kind="ExternalInput").ap()
    hist_out = nc.dram_tensor("ezhist", [128, L, HB], bf16, kind="ExternalOutput").ap()

    with TileContext(nc) as tc:
        with tc.tile_pool(name="const", bufs=1) as cpool, \
             tc.tile_pool(name="ep", bufs=2) as eppool, \
             tc.tile_pool(name="hist", bufs=1) as hpool, \
             tc.tile_pool(name="psv", bufs=4, space="PSUM") as vpool, \
             tc.tile_pool(name="pss", bufs=1, space="PSUM") as spool, \
             tc.tile_pool(name="warm", bufs=1, space="PSUM") as wpool:
            wbig = cpool.tile([128, 128], bf16)
            nc.gpsimd.dma_start(out=wbig[:], in_=w_in[:])
            scol = cpool.tile([128, 2], bf16)
            nc.gpsimd.dma_start(out=scol[:], in_=scol_in[:])
            bmask = cpool.tile([2, 128], bf16)
            nc.gpsimd.dma_start(out=bmask[:], in_=bmask_in[:])
            rs32 = cpool.tile([2, HB], f32)
            rsb = cpool.tile([2, HB], bf16)
            eps1 = cpool.tile([128, HB], bf16)
            hist = hpool.tile([128, L, HB], bf16)   # 32KB/partition

            # One-time PE warmup: ~NWARM back-to-back dummy matmuls (~4-5us of
            # continuous PE activity) flip the HAM clock gate to K=8/8; the
            # per-step gaps afterwards are far below the ~3.4us idle window,
            # so it never re-throttles. A memset source avoids any DMA
            # dependency, so warmup overlaps the input DMAs.
            wsrc = cpool.tile([128, 64], bf16)
            nc.any.memset(wsrc[:], 1.0)
            warm = wpool.tile([64, 64], f32)
            for _ in range(NWARM):
                nc.tensor.matmul(warm[:], wsrc[:], wsrc[:], start=True, stop=True)

            nchunks = L // CH
            for c in range(nchunks):
                ep = eppool.tile([128, CH, HB], bf16, tag="ep")
                nc.gpsimd.dma_start(out=ep[:], in_=ep_in[:, c * CH:(c + 1) * CH, :])

                if c == 0:
                    nc.scalar.copy(hist[:, 0, :], ep[:, 0, :])

                t0 = max(c * CH, 1)
                # Deferred kick tails: emitted 1 / 3 steps after the kick so
                # each op lands in an engine idle gap instead of stalling the
                # chain in the strict per-engine FIFOs.
                recip_at = {}   # t -> sp psum tile
                apply_at = {}   # t -> target ep slot for eps1
                for t in range(t0, (c + 1) * CH):
                    s = t - c * CH
                    vp = vpool.tile([128, HB], f32, tag="v")
                    nc.tensor.matmul(vp[:], wbig[:], hist[:, t - 1, :],
                                     start=True, stop=True)
                    # The renorm scale is pre-folded into eps1 for apply steps,
                    # so the serial chain is identical every step.
                    use_eps1 = t % G == LAG and t >= G + LAG
                    src1 = eps1[:] if use_eps1 else ep[:, s, :]
                    nc.vector.tensor_mul(hist[:, t, :], vp[:], src1)
                    if t in recip_at:
                        sp = recip_at.pop(t)
                        nc.vector.reciprocal(rs32[:], sp[:])
                        nc.scalar.copy(rsb[:], rs32[:])
                    if t in apply_at:
                        tgt = apply_at.pop(t)
                        bcp = spool.tile([128, HB], f32, tag="bc")
                        nc.tensor.matmul(bcp[:], bmask[:], rsb[:],
                                         start=True, stop=True)
                        nc.vector.tensor_mul(eps1[:], ep[:, tgt, :], bcp[:])
                    if t % G == 0 and G <= t < L - LAG:
                        # Off-chain renorm kick: S = per-stream sum_i EZ_t
                        # (rows 0/1 via the selector matmul); 1/S is broadcast
                        # with a K=2 mask matmul and folded into the EP slice
                        # of step t+LAG (always within the same chunk).
                        sp = spool.tile([2, HB], f32, tag="s")
                        nc.tensor.matmul(sp[:], scol[:], hist[:, t, :],
                                         start=True, stop=True)
                        recip_at[t + 1] = sp
                        apply_at[t + 3] = s + LAG

                nc.gpsimd.dma_start(
                    out=hist_out[:, c * CH:(c + 1) * CH, :],
                    in_=hist[:, c * CH:(c + 1) * CH, :],
                )

    nc.compile()
    _cache["nc"] = nc
    return nc


def _host_precompute(potentials, trans):
    """Per-core EP in [128, L, 16] bf16 stream layout + block-diag weights."""
    import ml_dtypes
    bf = ml_dtypes.bfloat16
    pm = potentials.max(axis=2, keepdims=True)
    EP = np.exp(KSC * (potentials - pm).astype(np.float64)).astype(bf)  # [B,L,T]
    eps = []
    for c in range(NCORES):
        epc = EP[c * BC:(c + 1) * BC]                    # [32, L, T]
        h = epc.reshape(2, HB, L, T).transpose(0, 3, 2, 1)  # [2, T, L, HB]
        eps.append(np.ascontiguousarray(h.reshape(128, L, HB)))
    expW = np.exp(KSC * trans.astype(np.float64)).astype(np.float32)    # [i, j]
    wbig = np.zeros((128, 128), dtype=bf)
    wbig[0:T, 0:T] = expW.astype(bf)
    wbig[T:128, T:128] = expW.astype(bf)
    scol = np.zeros((128, 2), dtype=bf)
    scol[0:T, 0] = 1
    scol[T:128, 1] = 1
    bmask = np.zeros((2, 128), dtype=bf)
    bmask[0, 0:T] = 1
    bmask[1, T:128] = 1
    return eps, wbig, scol, bmask, expW


def kernel(potentials, lengths, transition_params):
    from concourse.bass_utils import run_bass_kernel_spmd

    potentials = np.ascontiguousarray(np.asarray(potentials, dtype=np.float32))
    lengths = np.asarray(lengths, dtype=np.int32)
    trans = np.ascontiguousarray(np.asarray(transition_params, dtype=np.float32))

    nc = _build_program()
    eps, wbig, scol, bmask, expW = _host_precompute(potentials, trans)
    in_maps = [{"ep": eps[c], "wbig": wbig, "scol": scol, "bmask": bmask}
               for c in range(NCORES)]
    res = run_bass_kernel_spmd(nc, in_maps, core_ids=list(range(NCORES)))
    # [128, L, HB] per core -> EZ [B, L, T]
    parts = []
    for c in range(NCORES):
        arr = res.results[c]["ezhist"]                       # [128, L, HB] bf16
        h = arr.reshape(2, T, L, HB).transpose(0, 3, 2, 1)   # [2, HB, L, T]
        parts.append(h.reshape(BC, L, T))
    EZ = np.concatenate(parts, axis=0).astype(np.float64)    # [B, L, T]

    # Host backtrack in exp domain (monotone-equivalent to max-plus argmax).
    tags = np.zeros((B, L), dtype=np.int64)
    last = EZ[np.arange(B), lengths - 1, :].argmax(axis=1)
    tags[:, L - 1] = last
    lm1 = lengths - 1
    EW = expW.astype(np.float64)
    for t in range(L - 2, -1, -1):
        nxt = tags[:, t + 1]
        cand = EZ[:, t, :] * EW[:, nxt].T
        tags[:, t] = np.where(t >= lm1, last, cand.argmax(axis=1))
    return tags.astype(np.int32)
